# revision 15
# baseline (speedup 1.0000x reference)
"""Gaussian-mixture log-likelihood kernel for 8 Trainium2 NeuronCores.

Math: ll_i = logsumexp_j( -0.5 x_i^T A_j x_i + x_i^T m_j + bias_j ) - C
with A_j = S_j S_j^T.  The quadratic form is computed as ONE PE contraction of
577 rows per point: 544 symmetric-pair product rows packed as 17 circular
rotation blocks (row block o holds xT[i] * xT[(i+o)%32]), 32 x-rows for the
linear term, and one ones-row carrying the bias.  A global shift C (folded
into the bias on host) makes exp() safe without a per-point max.

Sharding: data-parallel over points, 16384 points/core; K-sized parameters
are replicated (precomputed on host in float64 — tiny vs the N*K work).
"""

import sys

sys.path.insert(0, "/opt/trn_rl_repo")

import numpy as np

import concourse.bass as bass
import bass_rust
import concourse.bacc as bacc
import concourse.mybir as mybir
from concourse import bass_utils
from concourse.bass_interp import get_hw_module
from concourse.tile import TileContext

N, K, D = 131072, 256, 32
NCORES = 8
NC_PTS = N // NCORES            # 16384 points per core
P = 1024                        # points per formation group
NGROUPS = NC_PTS // P           # 32
TPG = P // 128                  # point-tiles (128 pts) per group
NTILES = NC_PTS // 128          # 128 output columns
F32 = mybir.dt.float32
F32R = mybir.dt.float32r
F16 = mybir.dt.float16

_CACHE = {}


def _build(nc):
    ptsT = nc.dram_tensor("ptsT", [47, NC_PTS], F16, kind="ExternalInput").ap()
    aux = nc.dram_tensor("aux", [66, NC_PTS], F16, kind="ExternalInput").ap()
    bsym = nc.dram_tensor("bsym", [578, K], F16, kind="ExternalInput").ap()
    consts = nc.dram_tensor("consts", [128, 1], F32, kind="ExternalInput").ap()
    out = nc.dram_tensor("out", [128, NTILES], F32, kind="ExternalOutput").ap()

    with TileContext(nc) as tc:
        with (
            tc.tile_pool(name="rhs", bufs=1) as rhs_pool,
            tc.tile_pool(name="src", bufs=4) as src_pool,
            tc.tile_pool(name="x2t", bufs=4) as x2t_pool,
            tc.tile_pool(name="eps", bufs=3) as eps_pool,
            tc.tile_pool(name="acc", bufs=1) as acc_pool,
            tc.tile_pool(name="psum", bufs=8, space="PSUM") as psum_pool,
        ):
            # --- constants (loaded once) ---
            rhs = [rhs_pool.tile([128, K], F16, tag=f"rhs{c}", name=f"rhs{c}") for c in range(4)]
            rhs4 = rhs_pool.tile([128, K], F16, tag="rhs4")
            for c in range(4):
                nc.sync.dma_start(out=rhs[c][:, :], in_=bsym[128 * c:128 * (c + 1), :])
            nc.sync.dma_start(out=rhs4[0:66, :], in_=bsym[512:578, :])
            negC = rhs_pool.tile([128, 1], F32, tag="negC")
            nc.sync.dma_start(out=negC[:, :], in_=consts[:, :])

            s_all = acc_pool.tile([128, NTILES], F32, tag="s_all")
            ll_all = acc_pool.tile([128, NTILES], F32, tag="ll_all")

            for g in range(NGROUPS):
                lo = g * P
                hi = lo + P
                xid = src_pool.tile([128, P], F16, tag="xid")
                xrot = src_pool.tile([128, P], F16, tag="xrot")
                # xid: rows 0-31 replicated to 4 quadrants (0-stride source dim)
                nc.scalar.dma_start(out=xid[:, :],
                                    in_=ptsT[0:32, lo:hi].partition_broadcast(4))
                # xrot: quadrant a = rows a..a+31 (overlapping windows)
                xrot_src = bass_rust.AP(ptsT.tensor, lo,
                                        [(NC_PTS, 4), (NC_PTS, 32), (1, P)])
                nc.sync.dma_start(out=xrot[:, :], in_=xrot_src)

                x2t = [x2t_pool.tile([128, P], F16, tag=f"x2t{c}", name=f"x2t{c}") for c in range(4)]
                ch4 = x2t_pool.tile([128, P], F16, tag="ch4")
                r16 = src_pool.tile([32, P], F16, tag="r16")
                nc.scalar.dma_start(out=r16[:, :], in_=aux[0:32, lo:hi])
                nc.sync.dma_start(out=ch4[32:66, :], in_=aux[32:66, lo:hi])

                # chunk 0: rotation offsets 0..3 — xrot already is R_0
                nc.vector.tensor_mul(out=x2t[0][:, :], in0=xid[:, :], in1=xrot[:, :])
                for c in range(1, 4):
                    mask = [(i + 4 * c) % 32 for i in range(32)]
                    shf = src_pool.tile([128, P], F16, tag=f"shf{c}", name=f"shf{c}")
                    nc.vector.stream_shuffle(out=shf[:, :], in_=xrot[:, :], mask=mask)
                    eng = nc.gpsimd if c == 2 else nc.vector
                    eng.tensor_mul(out=x2t[c][:, :], in0=shf[:, :], in1=xid[:, :])
                # chunk4 rows 0-31: xT * rot16(xT)
                nc.gpsimd.tensor_mul(out=ch4[0:32, :], in0=r16[:, :], in1=xid[0:32, :])

                for t in range(TPG):
                    col = g * TPG + t
                    ts = slice(128 * t, 128 * (t + 1))
                    ps = psum_pool.tile([128, K], F32, tag="ps")
                    for j, c in enumerate((0, 1, 3, 2)):
                        nc.tensor.matmul(
                            out=ps[:, :],
                            lhsT=x2t[c][:, ts],
                            rhs=rhs[c][:, :],
                            start=(j == 0), stop=False,
                        )
                    nc.tensor.matmul(
                        out=ps[:, :],
                        lhsT=ch4[0:66, ts],
                        rhs=rhs4[0:66, :],
                        start=False, stop=True,
                    )
                    e_t = eps_pool.tile([128, K], F32, tag="e")
                    nc.scalar.activation(
                        out=e_t[:, :], in_=ps[:, :],
                        func=mybir.ActivationFunctionType.Exp,
                        accum_out=s_all[:, col:col + 1],
                    )

            # one Ln + one bias-add over all 128 columns (keeps ACT table warm)
            nc.scalar.activation(out=ll_all[:, :], in_=s_all[:, :],
                                 func=mybir.ActivationFunctionType.Ln)
            nc.vector.tensor_scalar_add(out=ll_all[:, :], in0=ll_all[:, :],
                                        scalar1=negC[:, 0:1])
            nc.sync.dma_start(out=out[:, :], in_=ll_all[:, :])
    return nc


def _get_module():
    if "nc" not in _CACHE:
        nc = bacc.Bacc("TRN2", target_bir_lowering=False, debug=False,
                       num_devices=NCORES)
        _build(nc)
        nc.compile()
        nc.m = get_hw_module(nc.m)
        _CACHE["nc"] = nc
    return _CACHE["nc"]


def _host_params(points, centers, covs_inv_sqrt, weights, threshold):
    S = covs_inv_sqrt.astype(np.float64)
    w = np.abs(weights.astype(np.float64))
    cp = w / (w.sum() + 1e-30)
    A = np.einsum("kde,kfe->kdf", S, S)
    _, logdetS = np.linalg.slogdet(S)
    logcoef = np.log(np.maximum(cp, 1e-300)) + logdetS  # + 0.5 * (2*logdetS)
    cen = centers.astype(np.float64)
    m = np.einsum("kde,ke->kd", A, cen)
    t_cAc = np.einsum("kd,kd->k", m, cen)
    thr = float(threshold[0])
    bias0 = logcoef - 0.5 * t_cAc - thr
    C = 4.0 - (logcoef.max() - thr)

    Brows = np.zeros((578, K))
    for c in range(4):
        for dl in range(4):
            o = 4 * c + dl
            q = 128 * c + 32 * dl
            for i in range(32):
                b = (i + o) % 32
                Brows[q + i] = (-0.5 * A[:, i, i]) if o == 0 else (-A[:, i, b])
    for i in range(32):
        Brows[512 + i] = -0.5 * A[:, i, (i + 16) % 32]
    Brows[544:576] = m.T
    bias = bias0 + C
    b_hi = bias.astype(np.float16).astype(np.float64)
    Brows[576] = b_hi
    Brows[577] = bias - b_hi
    return Brows.astype(np.float16), np.float32(-C)


def kernel(points, centers, covs_inv_sqrt, weights, threshold):
    points = np.asarray(points, dtype=np.float32)
    Brows, negC = _host_params(points, np.asarray(centers),
                               np.asarray(covs_inv_sqrt), np.asarray(weights),
                               np.asarray(threshold))
    consts = np.full((128, 1), negC, dtype=np.float32)

    in_maps = []
    for r in range(NCORES):
        pT = np.ascontiguousarray(points[r * NC_PTS:(r + 1) * NC_PTS].T)
        pT_ext = np.ascontiguousarray(
            np.vstack([pT, pT[:15]])).astype(np.float16)         # [47, Nc]
        ones = np.ones((2, NC_PTS), np.float16)
        aux = np.ascontiguousarray(
            np.vstack([pT[16:], pT[:16], pT, ones])).astype(np.float16)  # [66, Nc]
        in_maps.append({"ptsT": pT_ext, "aux": aux, "bsym": Brows, "consts": consts})

    nc = _get_module()
    res = bass_utils.run_bass_kernel_spmd(nc, in_maps,
                                          core_ids=list(range(NCORES)))
    ll = np.concatenate([res.results[r]["out"].T.reshape(-1)
                         for r in range(NCORES)])
    return ll.reshape(N, 1).astype(np.float32)
